# revision 1
# baseline (speedup 1.0000x reference)
"""Trainium2 Bass kernel for a pre-LN transformer encoder layer.

Problem shapes: x [4, 2048, 768], mask [4, 2048, 2048] (True = masked),
12 heads x 64 dh, FF 3072.

Sharding over 8 NeuronCores: core c handles batch b = c//2, query half
hf = c%2 (1024 queries). Each core recomputes LN1 + K/V for its whole
batch (cheap, avoids collectives); Q/attention/Wo/FF only for its half.
Host-side the sequence is permuted per core so the core's queries are
always tokens 0..1023 -> the SPMD program is identical on every core.

Layouts (SBUF):
  hT/QT/KT  [128 = dim-within-chunk, chunk, tokens]   (feature-major)
  V         [128 = token-within-chunk, kchunk, head, 65]  (ones col -> softmax sums)
  P=exp(S)  [128 = token-within-chunk(k), kchunk, q]
  ctxT      [64 = dh, head, q]
  x1        [128 = token-within-tile, tile, 768]      (token-major, fp32)

Matmul inputs are bf16 (same PE rate as fp32, half the SBUF/DMA), PSUM
accumulation fp32, residual path fp32. LN gamma/beta are folded into the
weights host-side (h@W with h = n*g + b  ==  n@(g*W) + b@W).

Schedule notes: the attention head loop is software-pipelined (PV of
head h-1 is emitted after QK/exp of head h) so the PE fills the
ACT-bound exp window; LN2 of each query group is emitted at that
group's attention tail so it overlaps the other group's attention.
"""

import numpy as np
import ml_dtypes

import concourse.bass as bass
import concourse.mybir as mybir
import concourse.tile as tile
from concourse import bacc
from concourse.bass_utils import run_bass_kernel_spmd
from concourse.masks import make_identity

F32 = mybir.dt.float32
BF16 = mybir.dt.bfloat16
AF = mybir.ActivationFunctionType
ALU = mybir.AluOpType

B, S, D, H, DH, FF = 4, 2048, 768, 12, 64, 3072
QL = S // 2          # queries per core
NT = S // 128        # 16 token tiles per batch
NQ = QL // 128       # 8 query tiles per core
FD = D // 128        # 6 feature chunks
MD = FF // 128       # 24 ff chunks
KC = S // 128        # 16 key chunks
EPS = 1e-5
N_CORES = 8

_BUILT = {}


def _ln_tile(nc, pools, src_ap, dst_bf16, epsb):
    """LayerNorm (no affine) of a [128, 768] fp32 AP -> bf16 tile."""
    stats, = pools
    st = stats.tile([128, 3, 6], F32, tag="bnst")
    for sg in range(3):
        nc.vector.bn_stats(st[:, sg, :], src_ap[:, sg * 256 : (sg + 1) * 256])
    mv = stats.tile([128, 2], F32, tag="mv")
    nc.vector.bn_aggr(mv, st)
    rstd = stats.tile([128, 1], F32, tag="rstd")
    nc.scalar.activation(rstd, mv[:, 1:2], AF.Sqrt, bias=epsb, scale=1.0)
    nc.vector.reciprocal(rstd, rstd)
    nc.vector.tensor_scalar(
        dst_bf16,
        src_ap,
        scalar1=mv[:, 0:1],
        scalar2=rstd,
        op0=ALU.subtract,
        op1=ALU.mult,
    )


def _emit(nc, io):
    tc_ctx = tile.TileContext(nc)
    with tc_ctx as tc:
        with (
            tc.tile_pool(name="const", bufs=1) as const,
            tc.tile_pool(name="x1p", bufs=1) as x1p,
            tc.tile_pool(name="h2Tp", bufs=1) as h2Tp,
        ):
            # ---- constants / persistent small tensors ----
            epsb = const.tile([128, 1], F32)
            nc.vector.memset(epsb, EPS)
            ident = const.tile([128, 128], F32)
            make_identity(nc, ident)
            identb = const.tile([128, 128], BF16)
            make_identity(nc, identb)
            shid = const.tile([64, 128], BF16)
            nc.sync.dma_start(shid, io["shid"][:])
            qb_sb = const.tile([128, FD], F32)
            nc.sync.dma_start(qb_sb, io["qb"][:])
            kb_sb = const.tile([128, FD], F32)
            nc.sync.dma_start(kb_sb, io["kb"][:])
            bob_sb = const.tile([128, FD], F32)
            nc.sync.dma_start(bob_sb, io["bob"][:])
            ub_sb = const.tile([128, MD], F32)
            nc.sync.dma_start(ub_sb, io["ub"][:])

            x1_sb = x1p.tile([128, NQ, D], F32)
            h2T_sb = h2Tp.tile([128, FD, QL], BF16)

            with (
                tc.tile_pool(name="attn_persist", bufs=1) as apers,
            ):
                QT_sb = apers.tile([128, FD, QL], BF16)
                KT_sb = apers.tile([128, FD, S], BF16)
                V_sb = apers.tile([128, KC, H, DH + 1], BF16)
                ctxT_sb = apers.tile([64, H, QL], BF16)

                # ================= Phase 1: LN1 + QKV =================
                with (
                    tc.tile_pool(name="wqkv", bufs=1) as wqkv,
                    tc.tile_pool(name="hTp", bufs=1) as hTp,
                    tc.tile_pool(name="p1s", bufs=3) as p1s,
                    tc.tile_pool(name="stats", bufs=6) as stats,
                    tc.tile_pool(name="p1_kq", bufs=3, space="PSUM") as kqp,
                    tc.tile_pool(name="p1_v", bufs=3, space="PSUM") as vpp,
                    tc.tile_pool(name="p1_tp", bufs=2, space="PSUM") as tpp1,
                ):
                    wv_sb = wqkv.tile([128, FD, D], BF16)
                    nc.sync.dma_start(wv_sb, io["wv"][:])
                    wq_sb = wqkv.tile([128, FD, D], BF16)
                    wk_sb = wqkv.tile([128, FD, D], BF16)
                    hT_sb = hTp.tile([128, FD, QL], BF16)
                    KT_own = hTp.tile([128, FD, QL], BF16)
                    V_own = hTp.tile([128, KC // 2, H, DH + 1], BF16)

                    nc.vector.memset(V_own[:, :, :, DH : DH + 1], 1.0)

                    xb_ap = io["xb"][:]
                    for t in range(NQ):
                        nc.sync.dma_start(
                            x1_sb[:, t, :], xb_ap[t * 128 : (t + 1) * 128, :]
                        )
                        h_t = p1s.tile([128, D], BF16, tag="ht")
                        _ln_tile(nc, (stats,), x1_sb[:, t, :], h_t, epsb)
                        for f in range(FD):
                            tpt = tpp1.tile([128, 128], BF16, tag="htp")
                            nc.tensor.transpose(
                                tpt, h_t[:, f * 128 : (f + 1) * 128], identb
                            )
                            nc.vector.tensor_copy(
                                hT_sb[:, f, t * 128 : (t + 1) * 128], tpt
                            )
                        # V for this token tile (token-major)
                        for n2 in range(2):
                            vps = vpp.tile([128, 384], F32, tag="vps")
                            for f in range(FD):
                                nc.tensor.matmul(
                                    vps,
                                    hT_sb[:, f, t * 128 : (t + 1) * 128],
                                    wv_sb[:, f, n2 * 384 : (n2 + 1) * 384],
                                    start=(f == 0),
                                    stop=(f == FD - 1),
                                )
                            nc.scalar.activation(
                                V_own[:, t, n2 * 6 : (n2 + 1) * 6, 0:DH],
                                vps.rearrange("p (h d) -> p h d", d=DH),
                                AF.Copy,
                            )
                        if t == 0:
                            nc.sync.dma_start(wk_sb, io["wk"][:])
                            nc.sync.dma_start(wq_sb, io["wq"][:])

                    # K over own half only; partner half arrives via AllGather
                    for g in range(2):
                        for m in range(FD):
                            kp = kqp.tile([128, 512], F32, tag="kqp")
                            for f in range(FD):
                                nc.tensor.matmul(
                                    kp,
                                    wk_sb[:, f, m * 128 : (m + 1) * 128],
                                    hT_sb[:, f, g * 512 : (g + 1) * 512],
                                    start=(f == 0),
                                    stop=(f == FD - 1),
                                )
                            nc.scalar.activation(
                                KT_own[:, m, g * 512 : (g + 1) * 512],
                                kp,
                                AF.Identity,
                                bias=kb_sb[:, m : m + 1],
                            )

                    # pack + pairwise AllGather; both ranks receive [A; B]
                    # (global key order), so the SPMD program stays uniform
                    KHALF = FD * QL
                    kv_in = io["kv_in"][:]
                    kv_ag = io["kv_ag"][:]
                    nc.sync.dma_start(
                        kv_in[:, 0:KHALF].rearrange("p (f q) -> p f q", f=FD),
                        KT_own,
                    )
                    nc.sync.dma_start(
                        kv_in[:, KHALF:].rearrange(
                            "p (k h d) -> p k h d", k=KC // 2, h=H
                        ),
                        V_own,
                    )
                    nc.gpsimd.collective_compute(
                        "AllGather",
                        mybir.AluOpType.bypass,
                        replica_groups=[[0, 1], [2, 3], [4, 5], [6, 7]],
                        ins=[kv_in.opt()],
                        outs=[kv_ag.opt()],
                    )
                    for half in range(2):
                        rows = slice(half * 128, (half + 1) * 128)
                        nc.sync.dma_start(
                            KT_sb[:, :, half * QL : (half + 1) * QL],
                            kv_ag[rows, 0:KHALF].rearrange(
                                "p (f q) -> p f q", f=FD
                            ),
                        )
                        nc.sync.dma_start(
                            V_sb[:, half * 8 : (half + 1) * 8, :, :],
                            kv_ag[rows, KHALF:].rearrange(
                                "p (k h d) -> p k h d", k=KC // 2, h=H
                            ),
                        )

                    # Q over own half (always tokens 0..1023 locally)
                    for g in range(2):
                        for m in range(FD):
                            qp = kqp.tile([128, 512], F32, tag="kqp")
                            for f in range(FD):
                                nc.tensor.matmul(
                                    qp,
                                    wq_sb[:, f, m * 128 : (m + 1) * 128],
                                    hT_sb[:, f, g * 512 : (g + 1) * 512],
                                    start=(f == 0),
                                    stop=(f == FD - 1),
                                )
                            nc.scalar.activation(
                                QT_sb[:, m, g * 512 : (g + 1) * 512],
                                qp,
                                AF.Identity,
                                bias=qb_sb[:, m : m + 1],
                            )

                # ======== Phase 2: attention (+ per-group LN2) ========
                mT_r = io["mT"][:].rearrange("(kc p) q -> p kc q", p=128)
                with (
                    tc.tile_pool(name="mqp", bufs=2) as mqp,
                    tc.tile_pool(name="pp", bufs=2) as pp,
                    tc.tile_pool(name="rbcp", bufs=2) as rbcp,
                    tc.tile_pool(name="attnTp", bufs=1) as attnTp,
                    tc.tile_pool(name="wop", bufs=2) as wop,
                    tc.tile_pool(name="p3s", bufs=2) as p3s,
                    tc.tile_pool(name="stats2", bufs=6) as stats2,
                    tc.tile_pool(name="p2_st", bufs=2, space="PSUM") as stpp,
                    tc.tile_pool(name="p2_ctx", bufs=2, space="PSUM") as ctxp,
                    tc.tile_pool(name="p2_tp", bufs=2, space="PSUM") as tpp,
                ):
                    for qg in range(2):
                        qs = slice(qg * 512, (qg + 1) * 512)
                        mqh = []
                        for half in range(2):
                            m_t = mqp.tile([128, KC // 2, 512], BF16, tag="mqh")
                            nc.sync.dma_start(
                                m_t, mT_r[:, half * 8 : (half + 1) * 8, qs]
                            )
                            mqh.append(m_t)

                        def qk_exp(h):
                            hp, hc = h % 2, h // 2
                            po = slice(hp * 64, hp * 64 + 64)
                            P_t = pp.tile([128, KC, 512], BF16, tag="P")
                            for kp2 in range(8):
                                stp_t = stpp.tile([128, 1024], F32, tag="st")
                                for j in range(2):
                                    kc = kp2 * 2 + j
                                    nc.tensor.matmul(
                                        stp_t[:, j * 512 : (j + 1) * 512],
                                        KT_sb[po, hc, kc * 128 : (kc + 1) * 128],
                                        QT_sb[po, hc, qs],
                                        start=True,
                                        stop=True,
                                    )
                                nc.scalar.activation(
                                    P_t[:, kp2 * 2 : kp2 * 2 + 2, :].rearrange(
                                        "p a b -> p (a b)"
                                    ),
                                    stp_t,
                                    AF.Exp,
                                    scale=0.125,
                                )
                            return P_t

                        def pv_ctx(h, P_t):
                            ctx_t = ctxp.tile([128, 512], F32, tag="ctx")
                            for half in range(2):
                                psl = P_t[:, half * 8 : (half + 1) * 8, :]
                                nc.vector.tensor_tensor(
                                    psl.rearrange("p a b -> p (a b)"),
                                    psl.rearrange("p a b -> p (a b)"),
                                    mqh[half].rearrange("p a b -> p (a b)"),
                                    ALU.mult,
                                )
                                for kc8 in range(KC // 2):
                                    kc = half * 8 + kc8
                                    nc.tensor.matmul(
                                        ctx_t[0:65, :],
                                        V_sb[:, kc, h, :],
                                        P_t[:, kc, :],
                                        start=(kc == 0),
                                        stop=(kc == KC - 1),
                                    )
                            nc.vector.tensor_copy(
                                ctxT_sb[:, h, qs], ctx_t[0:64, :]
                            )
                            # broadcast softmax sums (psum row 64) to 64
                            # partitions via a DRAM bounce (SBUF DMA sources
                            # reject zero partition step; DRAM allows it)
                            ssum = rbcp.tile([65, 512], F32, tag="ssum")
                            nc.vector.tensor_copy(
                                ssum[64:65, :], ctx_t[64:65, :]
                            )
                            nc.sync.dma_start(
                                io["rs_d"][:][qg * H + h, :], ssum[64:65, :]
                            )
                            sbc = rbcp.tile([64, 512], F32, tag="sbc")
                            rs_ap = io["rs_d"][:]
                            nc.sync.dma_start(
                                sbc,
                                bass.AP(
                                    tensor=rs_ap.tensor,
                                    offset=(qg * H + h) * 512,
                                    ap=[[0, 64], [1, 512]],
                                ),
                            )
                            nc.vector.reciprocal(sbc, sbc)
                            nc.vector.tensor_tensor(
                                ctxT_sb[:, h, qs], ctxT_sb[:, h, qs], sbc,
                                ALU.mult,
                            )

                        # software pipeline: QK/exp of head h overlaps PV of h-1
                        prev = None
                        for h in range(H):
                            P_t = qk_exp(h)
                            if prev is not None:
                                pv_ctx(prev[0], prev[1])
                            prev = (h, P_t)
                        pv_ctx(prev[0], prev[1])

                        # Wo + residual into x1: first stack head pairs to
                        # K=128 via identity-slice shift matmuls (exact), so Wo
                        # contracts with the full array instead of K=64
                        ctxP = attnTp.tile([128, FD, 512], BF16, tag="ctxP")
                        for c in range(FD):
                            pk = ctxp.tile([128, 512], F32, tag="ctx")
                            nc.tensor.matmul(
                                pk,
                                identb[0:64, :],
                                ctxT_sb[:, 2 * c, qs],
                                start=True,
                                stop=False,
                            )
                            nc.tensor.matmul(
                                pk,
                                shid,
                                ctxT_sb[:, 2 * c + 1, qs],
                                start=False,
                                stop=True,
                            )
                            nc.scalar.copy(ctxP[:, c, :], pk)
                        attnT_t = attnTp.tile([128, FD, 512], F32, tag="attnT")
                        for m in range(FD):
                            wob = wop.tile([128, FD, 128], BF16, tag="wob")
                            nc.sync.dma_start(wob, io["wo"][:][m])
                            ap_t = stpp.tile([128, 512], F32, tag="st")
                            for f in range(FD):
                                nc.tensor.matmul(
                                    ap_t,
                                    wob[:, f, :],
                                    ctxP[:, f, :],
                                    start=(f == 0),
                                    stop=(f == FD - 1),
                                )
                            nc.vector.tensor_scalar(
                                attnT_t[:, m, :],
                                ap_t,
                                scalar1=bob_sb[:, m : m + 1],
                                scalar2=None,
                                op0=ALU.add,
                            )
                        for m in range(FD):
                            for s4 in range(4):
                                tp_t = tpp.tile([128, 128], F32, tag="tp")
                                nc.tensor.transpose(
                                    tp_t,
                                    attnT_t[:, m, s4 * 128 : (s4 + 1) * 128],
                                    ident,
                                )
                                xsl = x1_sb[
                                    :, qg * 4 + s4, m * 128 : (m + 1) * 128
                                ]
                                nc.vector.tensor_tensor(xsl, xsl, tp_t, ALU.add)

                        # LN2 for this query group's tiles (overlaps the other
                        # group's attention)
                        for s4 in range(4):
                            s_ = qg * 4 + s4
                            h2_t = p3s.tile([128, D], BF16, tag="h2t")
                            _ln_tile(nc, (stats2,), x1_sb[:, s_, :], h2_t, epsb)
                            for f in range(FD):
                                tpt = tpp.tile([128, 128], BF16, tag="tp")
                                nc.tensor.transpose(
                                    tpt, h2_t[:, f * 128 : (f + 1) * 128], identb
                                )
                                nc.vector.tensor_copy(
                                    h2T_sb[:, f, s_ * 128 : (s_ + 1) * 128], tpt
                                )

            # ================= Phase 4: FF =================
            with (
                tc.tile_pool(name="uTp", bufs=1) as uTp,
                tc.tile_pool(name="w1p", bufs=3) as w1p,
                tc.tile_pool(name="w2p", bufs=2) as w2p,
                tc.tile_pool(name="ytp", bufs=2) as ytp,
                tc.tile_pool(name="p4_u", bufs=3, space="PSUM") as upp,
                tc.tile_pool(name="p4_y", bufs=2, space="PSUM") as ypp,
                tc.tile_pool(name="p4_tp", bufs=2, space="PSUM") as tpp4,
            ):
                uT_sb = uTp.tile([128, MD, QL], BF16)
                for m in range(MD):
                    w1b = w1p.tile([128, FD, 128], BF16, tag="w1b")
                    nc.sync.dma_start(w1b, io["w1"][:][m])
                    for qg in range(2):
                        qs = slice(qg * 512, (qg + 1) * 512)
                        up = upp.tile([128, 512], F32, tag="up")
                        for f in range(FD):
                            nc.tensor.matmul(
                                up,
                                w1b[:, f, :],
                                h2T_sb[:, f, qs],
                                start=(f == 0),
                                stop=(f == FD - 1),
                            )
                        nc.scalar.activation(
                            uT_sb[:, m, qs],
                            up,
                            AF.Relu,
                            bias=ub_sb[:, m : m + 1],
                        )
                for qg in range(2):
                    qs = slice(qg * 512, (qg + 1) * 512)
                    for m2 in range(FD):
                        w2b = w2p.tile([128, MD, 128], BF16, tag="w2b")
                        nc.sync.dma_start(w2b, io["w2"][:][m2])
                        yp = ypp.tile([128, 512], F32, tag="yp")
                        for f in range(MD):
                            nc.tensor.matmul(
                                yp,
                                w2b[:, f, :],
                                uT_sb[:, f, qs],
                                start=(f == 0),
                                stop=(f == MD - 1),
                            )
                        yt = ytp.tile([128, 512], F32, tag="yt")
                        nc.scalar.copy(yt, yp)
                        for s4 in range(4):
                            tp2 = tpp4.tile([128, 128], F32, tag="tp4")
                            nc.tensor.transpose(
                                tp2, yt[:, s4 * 128 : (s4 + 1) * 128], ident
                            )
                            xsl = x1_sb[
                                :, qg * 4 + s4, m2 * 128 : (m2 + 1) * 128
                            ]
                            nc.vector.tensor_tensor(xsl, xsl, tp2, ALU.add)
                    for s4 in range(4):
                        s_ = qg * 4 + s4
                        nc.sync.dma_start(
                            io["y"][:][s_ * 128 : (s_ + 1) * 128, :],
                            x1_sb[:, s_, :],
                        )


def _build():
    if "nc" in _BUILT:
        return _BUILT["nc"]
    nc = bacc.Bacc(None, target_bir_lowering=False, num_devices=8)
    io = {}
    io["xb"] = nc.dram_tensor("xb", [S, D], F32, kind="ExternalInput")
    io["mT"] = nc.dram_tensor("mT", [S, QL], BF16, kind="ExternalInput")
    io["wq"] = nc.dram_tensor("wq", [128, FD, D], BF16, kind="ExternalInput")
    io["wk"] = nc.dram_tensor("wk", [128, FD, D], BF16, kind="ExternalInput")
    io["wv"] = nc.dram_tensor("wv", [128, FD, D], BF16, kind="ExternalInput")
    io["wo"] = nc.dram_tensor("wo", [FD, 128, FD, 128], BF16, kind="ExternalInput")
    io["w1"] = nc.dram_tensor("w1", [MD, 128, FD, 128], BF16, kind="ExternalInput")
    io["w2"] = nc.dram_tensor("w2", [FD, 128, MD, 128], BF16, kind="ExternalInput")
    io["qb"] = nc.dram_tensor("qb", [128, FD], F32, kind="ExternalInput")
    io["kb"] = nc.dram_tensor("kb", [128, FD], F32, kind="ExternalInput")
    io["bob"] = nc.dram_tensor("bob", [128, FD], F32, kind="ExternalInput")
    io["ub"] = nc.dram_tensor("ub", [128, MD], F32, kind="ExternalInput")
    io["shid"] = nc.dram_tensor("shid", [64, 128], BF16, kind="ExternalInput")
    io["rs_d"] = nc.dram_tensor("rs_d", [2 * H, 512], F32)
    io["kv_in"] = nc.dram_tensor("kv_in", [128, 6 * 1024 + 8 * H * (DH + 1)], BF16)
    io["kv_ag"] = nc.dram_tensor(
        "kv_ag", [256, 6 * 1024 + 8 * H * (DH + 1)], BF16
    )
    io["y"] = nc.dram_tensor("y", [QL, D], F32, kind="ExternalOutput")
    _emit(nc, io)
    nc.compile()
    _BUILT["nc"] = nc
    return nc


def _host_prep(x, mask, wq, wk, wv, wo, bo, w1, w2, g1, b1, g2, b2):
    """Fold LN affine into weights, pre-chunk layouts, build per-core inputs."""
    bf = ml_dtypes.bfloat16
    x = np.asarray(x, np.float32)
    mask = np.asarray(mask, bool)
    g1 = np.asarray(g1, np.float32)
    b1 = np.asarray(b1, np.float32)
    g2 = np.asarray(g2, np.float32)
    b2 = np.asarray(b2, np.float32)
    wq = np.asarray(wq, np.float32)
    wk = np.asarray(wk, np.float32)
    wv = np.asarray(wv, np.float32)
    wo = np.asarray(wo, np.float32)
    w1 = np.asarray(w1, np.float32)
    w2 = np.asarray(w2, np.float32)
    bo = np.asarray(bo, np.float32)

    wq_g = g1[:, None] * wq
    wk_g = g1[:, None] * wk
    wv_g = g1[:, None] * wv
    w1_g = g2[:, None] * w1
    qb = b1 @ wq
    kb = b1 @ wk
    vb = b1 @ wv
    ub = b2 @ w1

    def chunk_k(w):  # [D, N] -> [128, FD, N]
        return np.ascontiguousarray(
            w.reshape(FD, 128, w.shape[1]).transpose(1, 0, 2).astype(bf)
        )

    wq_p = chunk_k(wq_g)
    wk_p = chunk_k(wk_g)
    # vb adds post-matmul and can't fold into wv; it's zero in this problem
    # (b1 is zeros) - assert so a nonzero bias can't silently skew V.
    assert np.abs(vb).max() == 0.0, "nonzero b1@wv not supported by this kernel"
    wv_p = chunk_k(wv_g)
    wo_p = np.ascontiguousarray(
        wo.reshape(FD, 128, FD, 128).transpose(2, 1, 0, 3).astype(bf)
    )  # [FD_m, 128, FD_f, 128]
    w1_p = np.ascontiguousarray(
        w1_g.reshape(FD, 128, MD, 128).transpose(2, 1, 0, 3).astype(bf)
    )  # [MD, 128, FD, 128]
    w2_p = np.ascontiguousarray(
        w2.reshape(MD, 128, FD, 128).transpose(2, 1, 0, 3).astype(bf)
    )  # [FD, 128, MD, 128]
    qb_p = np.ascontiguousarray(qb.reshape(FD, 128).T)
    kb_p = np.ascontiguousarray(kb.reshape(FD, 128).T)
    bob_p = np.ascontiguousarray(bo.reshape(FD, 128).T)
    ub_p = np.ascontiguousarray(ub.reshape(MD, 128).T)

    shid_p = np.ascontiguousarray(np.eye(64, 128, 64).astype(bf))
    shared = {
        "shid": shid_p,
        "wq": wq_p, "wk": wk_p, "wv": wv_p, "wo": wo_p,
        "w1": w1_p, "w2": w2_p,
        "qb": qb_p, "kb": kb_p, "bob": bob_p, "ub": ub_p,
    }
    in_maps = []
    for c in range(N_CORES):
        b, hf = c // 2, c % 2
        if hf == 0:
            xb = x[b]
        else:
            xb = np.concatenate([x[b, QL:], x[b, :QL]], axis=0)
        mrows = mask[b, hf * QL : (hf + 1) * QL, :]
        mT = np.ascontiguousarray(
            np.where(mrows, 0.0, 1.0).astype(bf).T
        )  # [S, QL] keys-major
        in_maps.append(
            {"xb": np.ascontiguousarray(xb), "mT": mT, **shared}
        )
    return in_maps


def kernel(**inputs):
    nc = _build()
    in_maps = _host_prep(**inputs)
    res = run_bass_kernel_spmd(nc, in_maps, list(range(N_CORES)))
    out = np.empty((B, S, D), np.float32)
    for c in range(N_CORES):
        b, hf = c // 2, c % 2
        out[b, hf * QL : (hf + 1) * QL] = res.results[c]["y"]
    return out

